# revision 21
# baseline (speedup 1.0000x reference)
"""Trainium2 Bass kernel for nn_Pool_12919261627034 (topk_masking).

Per batch b:
  col_sum = h[b].sum(0)                      # [D]
  scores  = h[b] @ col_sum                   # [N]
  idx     = sorted(indices of K smallest)    # [K]
  new_h   = h[b][idx]                        # [K, D]
  new_adj = adj[b][idx]                      # [K, N]

Sharding: data-parallel over batch — one batch per NeuronCore (8 cores).

Device algorithm per core:
  1. Stream h into SBUF; accumulate col_sum with PE matmuls (ones vector).
  2. Broadcast col_sum to 128 partitions with a PE matmul.
  3. scores[c*128+p] via fused DVE multiply + accumulate (scalar_tensor_tensor
     accum_out), one op per 128-row chunk.
  4. Binary search (36 unrolled iterations) on the score value t = K-th
     smallest: count(scores <= mid) via tensor_scalar(is_le, accum_out) +
     all-ones matmul (gives the total replicated on all 128 partitions).
  5. Exact top-K selection with index tie-break at the threshold:
     keep = (s < t) | (s == t & global_eq_prefix <= K - count_lt).
  6. Compact selected indices, in ascending order, with the gpsimd
     sparse_gather instruction ([16, F] f-major layout).
  7. Indirect-DMA gather of the selected h rows (2KB) and adj rows (16KB),
     128 rows per instruction, streamed back out to DRAM.
"""

import numpy as np

B = 8
N = 4096
D = 512
K = 2048
P = 128
NCHUNK = N // P          # 32
NGROUP = 8               # h DMA groups (4 chunks = 1MB each)
NROUND = 5               # 64-way CDF rounds (6 bits each)
NTAIL = 2                # binary-search safety iterations
RANGE0 = 16384.0         # initial binary search range (scores are ~±8000)

_cache = {}


def _build_nc():
    import concourse.bacc as bacc
    import concourse.bass as bass
    import concourse.mybir as mybir
    import concourse.tile as tile

    dt = mybir.dt
    Alu = mybir.AluOpType
    f32 = dt.float32

    nc = bacc.Bacc("TRN2", target_bir_lowering=False)

    h = nc.dram_tensor("h", [N, D], f32, kind="ExternalInput")
    adj = nc.dram_tensor("adj", [N, N], f32, kind="ExternalInput")
    new_h = nc.dram_tensor("new_h", [K, D], f32, kind="ExternalOutput")
    new_adj = nc.dram_tensor("new_adj", [K, N], f32, kind="ExternalOutput")
    idx_dbg = nc.dram_tensor("idx_dbg", [K], dt.int32, kind="ExternalOutput")
    nf_dbg = nc.dram_tensor("nf_dbg", [1, 1], dt.uint32, kind="ExternalOutput")

    # Constants (embedded in the NEFF).
    ones128_t = nc.inline_tensor(np.ones((P, P), np.float32), "ones128")
    # tri16[k, m] = 1 if k <= m  (inclusive prefix over partitions as lhsT)
    tri16_t = nc.inline_tensor(
        np.triu(np.ones((16, 16), np.float32), 0), "tri16"
    )
    # iota64[p, 0] = p % 64 + 1  (boundary index for the CDF rounds:
    # 64 boundaries, each counted over half the scores by two partitions)
    iota64_t = nc.inline_tensor(
        (np.arange(P, dtype=np.float32) % 64 + 1).reshape(P, 1).copy(), "iota64"
    )
    # W128[k, m] = 1 if k%64 == m%64: matmul merges the two half-counts of
    # each boundary and replicates the total onto all partitions.
    w = np.zeros((P, P), np.float32)
    for k in range(P):
        for m in range(P):
            if k % 64 == m % 64:
                w[k, m] = 1.0
    w128_t = nc.inline_tensor(w.astype(np.float16), "w128")
    halfones_t = nc.inline_tensor(
        np.full((P, P), 0.5, np.float16), "halfones"
    )
    # iota_p1[p, f] = (f*16 + p) + 1   (logical index n in f-major order, +1)
    iota_p1_t = nc.inline_tensor(
        (np.arange(N, dtype=np.float32).reshape(N // 16, 16).T + 1.0).copy(),
        "iota_p1",
    )

    with tile.TileContext(nc) as tc:
        with (
            tc.tile_pool(name="const", bufs=1) as constp,
            tc.tile_pool(name="hbig", bufs=1) as hbigp,
            tc.tile_pool(name="small", bufs=1) as smallp,
            tc.tile_pool(name="junk", bufs=2) as junkp,
            tc.tile_pool(name="adjrow", bufs=4) as adjp,
            tc.tile_pool(name="dram", bufs=1, space="DRAM") as dramp,
            tc.tile_pool(name="psum1", bufs=1, space="PSUM") as psum1,
            tc.tile_pool(name="psum2", bufs=1, space="PSUM") as psum2,
        ):
            # ---- constants to SBUF ----
            ones128 = constp.tile([P, P], f32)
            nc.sync.dma_start(out=ones128[:], in_=ones128_t[:, :])
            tri16 = constp.tile([16, 16], f32)
            nc.sync.dma_start(out=tri16[:], in_=tri16_t[:, :])
            iota_p1 = constp.tile([16, N // 16], f32)
            nc.sync.dma_start(out=iota_p1[:], in_=iota_p1_t[:, :])
            iota64 = constp.tile([P, 1], f32)
            nc.sync.dma_start(out=iota64[:], in_=iota64_t[:, :])
            w128 = constp.tile([P, P], dt.float16)
            nc.sync.dma_start(out=w128[:], in_=w128_t[:, :])
            halfones = constp.tile([P, P], dt.float16)
            nc.sync.dma_start(out=halfones[:], in_=halfones_t[:, :])

            # ---- 1. load h; running per-partition sum on DVE (overlaps
            #      the load), then one PE matmul contracts the partitions ----
            h_sb = hbigp.tile([P, NCHUNK * D], f32)
            acc = smallp.tile([P, D], f32)
            for g in range(NGROUP):
                cpg = NCHUNK // NGROUP  # chunks per group
                rows = cpg * P
                dma_eng = nc.sync if g % 2 == 0 else nc.scalar
                dma_eng.dma_start(
                    out=h_sb[:, g * cpg * D:(g + 1) * cpg * D].rearrange(
                        "p (c d) -> p c d", c=cpg
                    ),
                    in_=h[g * rows:(g + 1) * rows, :].rearrange(
                        "(c p) d -> p c d", p=P
                    ),
                )
                for i in range(cpg):
                    c = g * cpg + i
                    if c == 0:
                        nc.vector.tensor_copy(out=acc[:], in_=h_sb[:, :D])
                    else:
                        nc.vector.tensor_tensor(
                            out=acc[:], in0=acc[:],
                            in1=h_sb[:, c * D:(c + 1) * D], op=Alu.add,
                        )
            cs_psum = psum1.tile([1, D], f32, space="PSUM")
            nc.tensor.matmul(
                out=cs_psum[:], lhsT=ones128[:, :1], rhs=acc[:],
                start=True, stop=True,
            )
            cs_sb = smallp.tile([1, D], f32)
            nc.scalar.copy(out=cs_sb[:], in_=cs_psum[:])

            # ---- 2. broadcast col_sum to 128 partitions ----
            csb_psum = psum1.tile([P, D], f32, space="PSUM")
            nc.tensor.matmul(
                out=csb_psum[:], lhsT=ones128[:1, :], rhs=cs_sb[:],
                start=True, stop=True,
            )
            cs_b = smallp.tile([P, D], f32)
            nc.scalar.copy(out=cs_b[:], in_=csb_psum[:])

            # ---- 3. scores[c*128+p] = h row . col_sum ----
            scores_sb = smallp.tile([P, NCHUNK], f32)
            for c in range(NCHUNK):
                prod_junk = junkp.tile([P, D], f32)
                nc.vector.scalar_tensor_tensor(
                    out=prod_junk[:],
                    in0=h_sb[:, c * D:(c + 1) * D],
                    scalar=1.0,
                    in1=cs_b[:],
                    op0=Alu.mult,
                    op1=Alu.mult,
                    accum_out=scores_sb[:, c:c + 1],
                )

            # ---- bounce scores through DRAM (contiguous dump in p-major
            #      order: d[p*32 + c] = scores_sb[p, c] = score[c*128 + p]).
            #      All consumers re-derive the order on the read side with
            #      coarse-grained (>=128B-run) access patterns. ----
            scores_bounce = dramp.tile([N], f32)
            nc.sync.dma_start(
                out=scores_bounce[:].rearrange("(p c) -> p c", p=P),
                in_=scores_sb[:],
            )
            # s1625[q, f] = score[f*16 + q]: first load raw16[q, v*32+u] =
            # d[512v + 32q + u] (= score[128u + 16v + q]) with 128B-contiguous
            # DRAM runs, then one DVE copy permutes the free dim (f = 8u+v).
            raw16 = smallp.tile([16, N // 16], f32)
            nc.sync.dma_start(
                out=raw16[:].rearrange("q (v u) -> q v u", v=8),
                in_=scores_bounce[:].rearrange("(v q u) -> q v u", v=8, q=16),
            )
            s1625 = smallp.tile([16, N // 16], f32)
            nc.vector.tensor_copy(
                out=s1625[:].rearrange("q (u v) -> q u v", v=8),
                in_=raw16[:].rearrange("q (v u) -> q u v", v=8),
            )
            # s_all64: partitions 0-63 hold the first 2048 scores (dump
            # order), partitions 64-127 the second 2048.
            HN = N // 2
            s_all = smallp.tile([P, HN], f32)
            nc.sync.dma_start(
                out=s_all[:64, :],
                in_=scores_bounce[:HN].unsqueeze(0).to_broadcast([64, HN]),
            )
            nc.scalar.dma_start(
                out=s_all[64:, :],
                in_=scores_bounce[HN:].unsqueeze(0).to_broadcast([64, HN]),
            )

            # ---- 4a. 128-way CDF rounds: every partition p counts
            #      scores <= b_p for 128 evenly spaced boundaries at once,
            #      then the bracket index j = #(cnt_p < K) picks the new
            #      interval [b_j, b_j+1] -- 7 bits per round. ----
            lo = smallp.tile([P, 1], f32)
            hi = smallp.tile([P, 1], f32)
            mid = smallp.tile([P, 1], f32)
            part = smallp.tile([P, 1], f32)
            pred = smallp.tile([P, 1], dt.uint8)
            npred = smallp.tile([P, 1], dt.uint8)
            width = smallp.tile([P, 1], f32)
            step = smallp.tile([P, 1], f32)
            bnd = smallp.tile([P, 1], f32)
            cntr = smallp.tile([P, 1], dt.float16)
            mlo = smallp.tile([P, 1], dt.float16)
            jp1 = smallp.tile([P, 1], f32)
            hicand = smallp.tile([P, 1], f32)
            selhi = smallp.tile([P, 1], dt.uint8)
            nc.vector.memset(lo[:], -RANGE0)
            nc.vector.memset(hi[:], RANGE0)
            NB = 64  # boundaries per round
            for r in range(NROUND):
                cjunk = junkp.tile([P, N // 2], f32, tag="cjunk", bufs=1)
                nc.vector.tensor_tensor(
                    out=width[:], in0=hi[:], in1=lo[:], op=Alu.subtract
                )
                nc.vector.tensor_scalar_mul(step[:], width[:], 1.0 / NB)
                # b_p = fl(fl((p%64 + 1)*step) + lo)
                nc.vector.scalar_tensor_tensor(
                    out=bnd[:], in0=iota64[:], scalar=step[:], in1=lo[:],
                    op0=Alu.mult, op1=Alu.add,
                )
                # partition p counts its half of the scores vs b_p; the w128
                # matmul merges the two halves of each boundary and
                # replicates: jn = 2 * #(boundaries with count < K).
                nc.vector.tensor_scalar(
                    out=cjunk[:], in0=s_all[:], scalar1=bnd[:], scalar2=None,
                    op0=Alu.is_le, op1=Alu.add, accum_out=cntr[:],
                )
                jn_psum = psum2.tile([P, 1], f32, space="PSUM", tag="cnt")
                nc.tensor.matmul(
                    out=jn_psum[:], lhsT=w128[:], rhs=cntr[:],
                    start=True, stop=True,
                )
                nc.vector.tensor_scalar(
                    out=mlo[:], in0=jn_psum[:], scalar1=float(K), scalar2=None,
                    op0=Alu.is_lt,
                )
                jhalf_psum = psum2.tile([P, 1], f32, space="PSUM", tag="cnt2")
                nc.tensor.matmul(
                    out=jhalf_psum[:], lhsT=halfones[:], rhs=mlo[:],
                    start=True, stop=True,
                )
                # hi' = b_(j+1) if j < 64 else hi; lo' = b_j (bit-identical
                # fl ops to the bnd computation keep the invariant exact).
                nc.vector.tensor_scalar(
                    out=jp1[:], in0=jhalf_psum[:], scalar1=1.0, scalar2=None,
                    op0=Alu.add,
                )
                nc.vector.scalar_tensor_tensor(
                    out=hicand[:], in0=jp1[:], scalar=step[:], in1=lo[:],
                    op0=Alu.mult, op1=Alu.add,
                )
                nc.vector.tensor_scalar(
                    out=selhi[:], in0=jhalf_psum[:], scalar1=float(NB),
                    scalar2=None, op0=Alu.is_lt,
                )
                nc.vector.scalar_tensor_tensor(
                    out=lo[:], in0=jhalf_psum[:], scalar=step[:], in1=lo[:],
                    op0=Alu.mult, op1=Alu.add,
                )
                nc.vector.copy_predicated(
                    out=hi[:], mask=selhi[:], data=hicand[:]
                )

            # ---- 4b. binary tail (safety net; no-op once adjacent) ----
            for it in range(NTAIL):
                bs_junk = junkp.tile([P, NCHUNK], f32, tag="bsjunk")
                nc.vector.tensor_tensor(
                    out=mid[:], in0=lo[:], in1=hi[:], op=Alu.add
                )
                nc.vector.tensor_scalar_mul(mid[:], mid[:], 0.5)
                nc.vector.tensor_scalar(
                    out=bs_junk[:],
                    in0=scores_sb[:],
                    scalar1=mid[:],
                    scalar2=None,
                    op0=Alu.is_le,
                    op1=Alu.add,
                    accum_out=part[:],
                )
                cnt_psum = psum2.tile([P, 1], f32, space="PSUM", tag="cnt")
                nc.tensor.matmul(
                    out=cnt_psum[:], lhsT=ones128[:], rhs=part[:],
                    start=True, stop=True,
                )
                nc.vector.tensor_scalar(
                    out=pred[:], in0=cnt_psum[:], scalar1=float(K),
                    scalar2=None, op0=Alu.is_ge,
                )
                nc.vector.tensor_scalar(
                    out=npred[:], in0=cnt_psum[:], scalar1=float(K),
                    scalar2=None, op0=Alu.is_lt,
                )
                nc.vector.copy_predicated(out=hi[:], mask=pred[:], data=mid[:])
                nc.vector.copy_predicated(out=lo[:], mask=npred[:], data=mid[:])

            # ---- 5. exact selection mask with index tie-break ----
            thr16 = hi[:16, :1]
            F = N // 16  # 256
            mlt = smallp.tile([16, F], f32)
            red_lt = smallp.tile([16, 1], f32)
            nc.vector.tensor_scalar(
                out=mlt[:], in0=s1625[:], scalar1=thr16, scalar2=None,
                op0=Alu.is_lt, op1=Alu.add, accum_out=red_lt[:],
            )
            cnt16_psum = psum1.tile([16, 1], f32, space="PSUM")
            nc.tensor.matmul(
                out=cnt16_psum[:], lhsT=ones128[:16, :16],
                rhs=red_lt[:], start=True, stop=True,
            )
            # need = K - count_lt   (count of == t entries to keep)
            need16 = smallp.tile([16, 1], f32)
            nc.vector.tensor_scalar(
                out=need16[:], in0=cnt16_psum[:], scalar1=-1.0,
                scalar2=float(K), op0=Alu.mult, op1=Alu.add,
            )
            meq = smallp.tile([16, F], f32)
            nc.vector.tensor_scalar(
                out=meq[:], in0=s1625[:], scalar1=thr16, scalar2=None,
                op0=Alu.is_equal,
            )
            # global inclusive prefix of meq in f-major (logical n) order:
            #   colsum[f]  = sum_p meq[p, f]          (replicated, PE)
            #   colcum[f]  = inclusive scan_f colsum  (DVE scan)
            #   partial    = sum_{p'<=p} meq[p', f]   (PE, tri16 lhsT)
            #   prefix     = colcum - colsum + partial
            colsum_psum = psum1.tile([16, F], f32, space="PSUM")
            nc.tensor.matmul(
                out=colsum_psum[:], lhsT=ones128[:16, :16],
                rhs=meq[:], start=True, stop=True,
            )
            zeros16 = smallp.tile([16, F], f32)
            nc.vector.memset(zeros16[:], 0.0)
            colcum = smallp.tile([16, F], f32)
            nc.vector.tensor_tensor_scan(
                out=colcum[:], data0=zeros16[:], data1=colsum_psum[:],
                initial=0.0, op0=Alu.add, op1=Alu.add,
            )
            partial_psum = psum1.tile([16, F], f32, space="PSUM")
            nc.tensor.matmul(
                out=partial_psum[:], lhsT=tri16[:], rhs=meq[:],
                start=True, stop=True,
            )
            excl = smallp.tile([16, F], f32)
            nc.vector.tensor_tensor(
                out=excl[:], in0=colcum[:], in1=colsum_psum[:], op=Alu.subtract
            )
            prefix = smallp.tile([16, F], f32)
            nc.vector.tensor_tensor(
                out=prefix[:], in0=excl[:], in1=partial_psum[:], op=Alu.add
            )
            keepeq = smallp.tile([16, F], f32)
            nc.vector.tensor_scalar(
                out=keepeq[:], in0=prefix[:], scalar1=need16[:], scalar2=None,
                op0=Alu.is_le,
            )
            # select only where s == t:
            keepeq2 = smallp.tile([16, F], f32)
            nc.vector.tensor_tensor(
                out=keepeq2[:], in0=keepeq[:], in1=meq[:], op=Alu.mult
            )
            fmask = smallp.tile([16, F], f32)
            nc.vector.tensor_tensor(
                out=fmask[:], in0=mlt[:], in1=keepeq2[:], op=Alu.add
            )
            # ---- slot table for the new_h scatter:
            #      slot(n) = (inclusive prefix of fmask at n) - 1 if selected
            #      else 4095 (> bounds_check -> row skipped) ----
            colsumF_psum = psum1.tile(
                [16, F], f32, space="PSUM", tag="colsum_psum"
            )
            nc.tensor.matmul(
                out=colsumF_psum[:], lhsT=ones128[:16, :16], rhs=fmask[:],
                start=True, stop=True,
            )
            colcumF = smallp.tile([16, F], f32)
            nc.vector.tensor_tensor_scan(
                out=colcumF[:], data0=zeros16[:], data1=colsumF_psum[:],
                initial=0.0, op0=Alu.add, op1=Alu.add,
            )
            partialF_psum = psum1.tile(
                [16, F], f32, space="PSUM", tag="partial_psum"
            )
            nc.tensor.matmul(
                out=partialF_psum[:], lhsT=tri16[:], rhs=fmask[:],
                start=True, stop=True,
            )
            exclF = smallp.tile([16, F], f32)
            nc.vector.tensor_tensor(
                out=exclF[:], in0=colcumF[:], in1=colsumF_psum[:],
                op=Alu.subtract,
            )
            prefF = smallp.tile([16, F], f32)
            nc.vector.tensor_tensor(
                out=prefF[:], in0=exclF[:], in1=partialF_psum[:], op=Alu.add
            )
            slotp = smallp.tile([16, F], f32)
            nc.vector.tensor_tensor(
                out=slotp[:], in0=prefF[:], in1=fmask[:], op=Alu.mult
            )
            slotf = smallp.tile([16, F], f32)
            nc.vector.scalar_tensor_tensor(
                out=slotf[:], in0=fmask[:], scalar=-4096.0, in1=slotp[:],
                op0=Alu.mult, op1=Alu.add,
            )
            nc.vector.tensor_scalar(
                out=slotf[:], in0=slotf[:], scalar1=4095.0, scalar2=None,
                op0=Alu.add,
            )
            # un-permute to raw order, cast, dump p-major, reload [128, 32]
            rawS = smallp.tile([16, F], f32)
            nc.vector.tensor_copy(
                out=rawS[:].rearrange("q (v u) -> q u v", v=8),
                in_=slotf[:].rearrange("q (u v) -> q u v", v=8),
            )
            rawS_i32 = smallp.tile([16, F], dt.int32)
            nc.vector.tensor_copy(out=rawS_i32[:], in_=rawS[:])
            slot_bounce = dramp.tile([N], dt.int32)
            nc.scalar.dma_start(
                out=slot_bounce[:].rearrange("(v q u) -> q v u", v=8, q=16),
                in_=rawS_i32[:].rearrange("q (v u) -> q v u", v=8),
            )
            slots_sb = smallp.tile([P, NCHUNK], dt.int32)
            nc.scalar.dma_start(
                out=slots_sb[:],
                in_=slot_bounce[:].rearrange("(p c) -> p c", p=P),
            )

            # masked_idx = fmask * (n+1) - 1   (selected -> n, else -1)
            masked = smallp.tile([16, F], f32)
            nc.vector.tensor_tensor(
                out=masked[:], in0=fmask[:], in1=iota_p1[:], op=Alu.mult
            )
            nc.vector.tensor_scalar(
                out=masked[:], in0=masked[:], scalar1=-1.0, scalar2=None,
                op0=Alu.add,
            )

            # ---- 6. compact to sorted index list ----
            compact = smallp.tile([16, K // 16], f32)
            nf = smallp.tile([1, 1], dt.uint32)
            nc.gpsimd.sparse_gather(
                out=compact[:], in_=masked[:], num_found=nf[:]
            )
            nc.sync.dma_start(out=nf_dbg[:, :], in_=nf[:])
            compact_i32 = smallp.tile([16, K // 16], dt.int32)
            nc.vector.tensor_copy(out=compact_i32[:], in_=compact[:])
            # Contiguous dump: d2[q*128 + f] = idx[f*16 + q]. Column m of
            # d2 (contiguous 512B) holds idx[j*16 + m] for j = 0..127, so
            # gather instruction m writes output rows j*16 + m (a stride-16
            # row view -- still whole 16KB/2KB rows per descriptor).
            idx_bounce = dramp.tile([K], dt.int32)
            nc.sync.dma_start(
                out=idx_bounce[:].rearrange("(q f) -> q f", q=16),
                in_=compact_i32[:],
            )
            nc.sync.dma_start(
                out=idx_dbg.rearrange("(q f) -> q f", q=16),
                in_=compact_i32[:],
            )
            adj_rows = new_adj.rearrange("(f q) w -> q f w", q=16)
            idxcols = []
            for m in range(16):
                idxcol = smallp.tile(
                    [P, 1], dt.int32, tag=f"idxcol{m}", name=f"idxcol{m}"
                )
                eng = nc.sync if m % 2 == 0 else nc.scalar
                eng.dma_start(
                    out=idxcol[:], in_=idx_bounce[m * P:(m + 1) * P, None]
                )
                idxcols.append(idxcol)
            for m in range(16):
                adjrow = adjp.tile([P, N], f32)
                nc.gpsimd.indirect_dma_start(
                    out=adjrow[:],
                    out_offset=None,
                    in_=adj[:, :],
                    in_offset=bass.IndirectOffsetOnAxis(
                        ap=idxcols[m][:], axis=0
                    ),
                )
                eng = nc.sync if m % 2 == 0 else nc.scalar
                eng.dma_start(out=adj_rows[m], in_=adjrow[:])
            for c in range(NCHUNK):
                nc.gpsimd.indirect_dma_start(
                    out=new_h[:, :],
                    out_offset=bass.IndirectOffsetOnAxis(
                        ap=slots_sb[:, c:c + 1], axis=0
                    ),
                    in_=h_sb[:, c * D:(c + 1) * D],
                    in_offset=None,
                    bounds_check=K - 1,
                    oob_is_err=False,
                )

    nc.compile()
    return nc


def _get_nc():
    if "nc" not in _cache:
        _cache["nc"] = _build_nc()
    return _cache["nc"]


def kernel(h, adj):
    from concourse.bass_utils import run_bass_kernel_spmd

    h = np.ascontiguousarray(np.asarray(h), dtype=np.float32)
    adj = np.ascontiguousarray(np.asarray(adj), dtype=np.float32)
    assert h.shape == (B, N, D) and adj.shape == (B, N, N)

    nc = _get_nc()
    in_maps = [{"h": h[b], "adj": adj[b]} for b in range(B)]
    res = run_bass_kernel_spmd(nc, in_maps, core_ids=list(range(B)))
    new_h = np.stack([res.results[b]["new_h"] for b in range(B)])
    new_adj = np.stack([res.results[b]["new_adj"] for b in range(B)])
    return new_h, new_adj


# revision 23
# speedup vs baseline: 1.2057x; 1.2057x over previous
"""Trainium2 Bass kernel for nn_Pool_12919261627034 (topk_masking).

Per batch b:
  col_sum = h[b].sum(0)                      # [D]
  scores  = h[b] @ col_sum                   # [N]
  idx     = sorted(indices of K smallest)    # [K]
  new_h   = h[b][idx]                        # [K, D]
  new_adj = adj[b][idx]                      # [K, N]

Sharding: data-parallel over batch — one batch per NeuronCore (8 cores).

Device algorithm per core:
  1. Stream h into SBUF; accumulate col_sum with PE matmuls (ones vector).
  2. Broadcast col_sum to 128 partitions with a PE matmul.
  3. scores[c*128+p] via fused DVE multiply + accumulate (scalar_tensor_tensor
     accum_out), one op per 128-row chunk.
  4. Binary search (36 unrolled iterations) on the score value t = K-th
     smallest: count(scores <= mid) via tensor_scalar(is_le, accum_out) +
     all-ones matmul (gives the total replicated on all 128 partitions).
  5. Exact top-K selection with index tie-break at the threshold:
     keep = (s < t) | (s == t & global_eq_prefix <= K - count_lt).
  6. Compact selected indices, in ascending order, with the gpsimd
     sparse_gather instruction ([16, F] f-major layout).
  7. Indirect-DMA gather of the selected h rows (2KB) and adj rows (16KB),
     128 rows per instruction, streamed back out to DRAM.
"""

from contextlib import ExitStack

import numpy as np

B = 8
N = 4096
D = 512
K = 2048
P = 128
NCHUNK = N // P          # 32
NGROUP = 8               # h DMA groups (4 chunks = 1MB each)
NROUND = 5               # 64-way CDF rounds (6 bits each)
NTAIL = 2                # binary-search safety iterations
RANGE0 = 16384.0         # initial binary search range (scores are ~±8000)

_cache = {}


def _build_nc():
    import concourse.bacc as bacc
    import concourse.bass as bass
    import concourse.mybir as mybir
    import concourse.tile as tile

    dt = mybir.dt
    Alu = mybir.AluOpType
    f32 = dt.float32

    nc = bacc.Bacc("TRN2", target_bir_lowering=False)

    h = nc.dram_tensor("h", [N, D], f32, kind="ExternalInput")
    adj = nc.dram_tensor("adj", [N, N], f32, kind="ExternalInput")
    new_h = nc.dram_tensor("new_h", [K, D], f32, kind="ExternalOutput")
    new_adj = nc.dram_tensor("new_adj", [K, N], f32, kind="ExternalOutput")
    idx_dbg = nc.dram_tensor("idx_dbg", [K], dt.int32, kind="ExternalOutput")
    nf_dbg = nc.dram_tensor("nf_dbg", [1, 1], dt.uint32, kind="ExternalOutput")

    # Constants (embedded in the NEFF).
    ones128_t = nc.inline_tensor(np.ones((P, P), np.float32), "ones128")
    # tri16[k, m] = 1 if k <= m  (inclusive prefix over partitions as lhsT)
    tri16_t = nc.inline_tensor(
        np.triu(np.ones((16, 16), np.float32), 0), "tri16"
    )
    # iota64[p, 0] = p % 64 + 1  (boundary index for the CDF rounds:
    # 64 boundaries, each counted over half the scores by two partitions)
    iota64_t = nc.inline_tensor(
        (np.arange(P, dtype=np.float32) % 64 + 1).reshape(P, 1).copy(), "iota64"
    )
    # W128[k, m] = 1 if k%64 == m%64: matmul merges the two half-counts of
    # each boundary and replicates the total onto all partitions.
    w = np.zeros((P, P), np.float32)
    for k in range(P):
        for m in range(P):
            if k % 64 == m % 64:
                w[k, m] = 1.0
    w128_t = nc.inline_tensor(w.astype(np.float16), "w128")
    halfones_t = nc.inline_tensor(
        np.full((P, P), 0.5, np.float16), "halfones"
    )
    # iota_p1[p, f] = (f*16 + p) + 1   (logical index n in f-major order, +1)
    iota_p1_t = nc.inline_tensor(
        (np.arange(N, dtype=np.float32).reshape(N // 16, 16).T + 1.0).copy(),
        "iota_p1",
    )

    with tile.TileContext(nc) as tc:
        with (
            tc.tile_pool(name="const", bufs=1) as constp,
            tc.tile_pool(name="small", bufs=1) as smallp,
            tc.tile_pool(name="junk", bufs=2) as junkp,
            tc.tile_pool(name="adjrow", bufs=5) as adjp,
            tc.tile_pool(name="hrow", bufs=4) as hrowp,
            tc.tile_pool(name="dram", bufs=1, space="DRAM") as dramp,
            tc.tile_pool(name="psum1", bufs=1, space="PSUM") as psum1,
            tc.tile_pool(name="psum2", bufs=1, space="PSUM") as psum2,
        ):
            # ---- constants to SBUF ----
            ones128 = constp.tile([P, P], f32)
            nc.sync.dma_start(out=ones128[:], in_=ones128_t[:, :])
            tri16 = constp.tile([16, 16], f32)
            nc.sync.dma_start(out=tri16[:], in_=tri16_t[:, :])
            iota_p1 = constp.tile([16, N // 16], f32)
            nc.sync.dma_start(out=iota_p1[:], in_=iota_p1_t[:, :])
            iota64 = constp.tile([P, 1], f32)
            nc.sync.dma_start(out=iota64[:], in_=iota64_t[:, :])
            w128 = constp.tile([P, P], dt.float16)
            nc.sync.dma_start(out=w128[:], in_=w128_t[:, :])
            halfones = constp.tile([P, P], dt.float16)
            nc.sync.dma_start(out=halfones[:], in_=halfones_t[:, :])

            # ---- 1. load h; running per-partition sum on DVE (overlaps
            #      the load), then one PE matmul contracts the partitions ----
            _hbig_es = ExitStack()
            hbigp = _hbig_es.enter_context(tc.tile_pool(name="hbig", bufs=1))
            h_sb = hbigp.tile([P, NCHUNK * D], f32)
            acc = smallp.tile([P, D], f32)
            for g in range(NGROUP):
                cpg = NCHUNK // NGROUP  # chunks per group
                rows = cpg * P
                dma_eng = nc.sync if g % 2 == 0 else nc.scalar
                dma_eng.dma_start(
                    out=h_sb[:, g * cpg * D:(g + 1) * cpg * D].rearrange(
                        "p (c d) -> p c d", c=cpg
                    ),
                    in_=h[g * rows:(g + 1) * rows, :].rearrange(
                        "(c p) d -> p c d", p=P
                    ),
                )
                for i in range(cpg):
                    c = g * cpg + i
                    if c == 0:
                        nc.vector.tensor_copy(out=acc[:], in_=h_sb[:, :D])
                    else:
                        nc.vector.tensor_tensor(
                            out=acc[:], in0=acc[:],
                            in1=h_sb[:, c * D:(c + 1) * D], op=Alu.add,
                        )
            cs_psum = psum1.tile([1, D], f32, space="PSUM")
            nc.tensor.matmul(
                out=cs_psum[:], lhsT=ones128[:, :1], rhs=acc[:],
                start=True, stop=True,
            )
            cs_sb = smallp.tile([1, D], f32)
            nc.scalar.copy(out=cs_sb[:], in_=cs_psum[:])

            # ---- 2. broadcast col_sum to 128 partitions ----
            csb_psum = psum1.tile([P, D], f32, space="PSUM")
            nc.tensor.matmul(
                out=csb_psum[:], lhsT=ones128[:1, :], rhs=cs_sb[:],
                start=True, stop=True,
            )
            cs_b = smallp.tile([P, D], f32)
            nc.scalar.copy(out=cs_b[:], in_=csb_psum[:])

            # ---- 3. scores[c*128+p] = h row . col_sum ----
            scores_sb = smallp.tile([P, NCHUNK], f32)
            for c in range(NCHUNK):
                prod_junk = junkp.tile([P, D], f32)
                nc.vector.scalar_tensor_tensor(
                    out=prod_junk[:],
                    in0=h_sb[:, c * D:(c + 1) * D],
                    scalar=1.0,
                    in1=cs_b[:],
                    op0=Alu.mult,
                    op1=Alu.mult,
                    accum_out=scores_sb[:, c:c + 1],
                )

            _hbig_es.close()

            # ---- bounce scores through DRAM (contiguous dump in p-major
            #      order: d[p*32 + c] = scores_sb[p, c] = score[c*128 + p]).
            #      All consumers re-derive the order on the read side with
            #      coarse-grained (>=128B-run) access patterns. ----
            scores_bounce = dramp.tile([N], f32)
            nc.sync.dma_start(
                out=scores_bounce[:].rearrange("(p c) -> p c", p=P),
                in_=scores_sb[:],
            )
            # s1625[q, f] = score[f*16 + q]: first load raw16[q, v*32+u] =
            # d[512v + 32q + u] (= score[128u + 16v + q]) with 128B-contiguous
            # DRAM runs, then one DVE copy permutes the free dim (f = 8u+v).
            raw16 = smallp.tile([16, N // 16], f32)
            nc.sync.dma_start(
                out=raw16[:].rearrange("q (v u) -> q v u", v=8),
                in_=scores_bounce[:].rearrange("(v q u) -> q v u", v=8, q=16),
            )
            s1625 = smallp.tile([16, N // 16], f32)
            nc.vector.tensor_copy(
                out=s1625[:].rearrange("q (u v) -> q u v", v=8),
                in_=raw16[:].rearrange("q (v u) -> q u v", v=8),
            )
            # s_all64: partitions 0-63 hold the first 2048 scores (dump
            # order), partitions 64-127 the second 2048.
            HN = N // 2
            s_all = smallp.tile([P, HN], f32)
            nc.sync.dma_start(
                out=s_all[:64, :],
                in_=scores_bounce[:HN].unsqueeze(0).to_broadcast([64, HN]),
            )
            nc.scalar.dma_start(
                out=s_all[64:, :],
                in_=scores_bounce[HN:].unsqueeze(0).to_broadcast([64, HN]),
            )

            # ---- 4a. 128-way CDF rounds: every partition p counts
            #      scores <= b_p for 128 evenly spaced boundaries at once,
            #      then the bracket index j = #(cnt_p < K) picks the new
            #      interval [b_j, b_j+1] -- 7 bits per round. ----
            lo = smallp.tile([P, 1], f32)
            hi = smallp.tile([P, 1], f32)
            mid = smallp.tile([P, 1], f32)
            part = smallp.tile([P, 1], f32)
            pred = smallp.tile([P, 1], dt.uint8)
            npred = smallp.tile([P, 1], dt.uint8)
            width = smallp.tile([P, 1], f32)
            step = smallp.tile([P, 1], f32)
            bnd = smallp.tile([P, 1], f32)
            cntr = smallp.tile([P, 1], dt.float16)
            mlo = smallp.tile([P, 1], dt.float16)
            jp1 = smallp.tile([P, 1], f32)
            hicand = smallp.tile([P, 1], f32)
            selhi = smallp.tile([P, 1], dt.uint8)
            nc.vector.memset(lo[:], -RANGE0)
            nc.vector.memset(hi[:], RANGE0)
            NB = 64  # boundaries per round
            for r in range(NROUND):
                cjunk = junkp.tile([P, N // 2], f32, tag="cjunk", bufs=1)
                nc.vector.tensor_tensor(
                    out=width[:], in0=hi[:], in1=lo[:], op=Alu.subtract
                )
                nc.vector.tensor_scalar_mul(step[:], width[:], 1.0 / NB)
                # b_p = fl(fl((p%64 + 1)*step) + lo)
                nc.vector.scalar_tensor_tensor(
                    out=bnd[:], in0=iota64[:], scalar=step[:], in1=lo[:],
                    op0=Alu.mult, op1=Alu.add,
                )
                # partition p counts its half of the scores vs b_p; the w128
                # matmul merges the two halves of each boundary and
                # replicates: jn = 2 * #(boundaries with count < K).
                nc.vector.tensor_scalar(
                    out=cjunk[:], in0=s_all[:], scalar1=bnd[:], scalar2=None,
                    op0=Alu.is_le, op1=Alu.add, accum_out=cntr[:],
                )
                jn_psum = psum2.tile([P, 1], f32, space="PSUM", tag="cnt")
                nc.tensor.matmul(
                    out=jn_psum[:], lhsT=w128[:], rhs=cntr[:],
                    start=True, stop=True,
                )
                nc.vector.tensor_scalar(
                    out=mlo[:], in0=jn_psum[:], scalar1=float(K), scalar2=None,
                    op0=Alu.is_lt,
                )
                jhalf_psum = psum2.tile([P, 1], f32, space="PSUM", tag="cnt2")
                nc.tensor.matmul(
                    out=jhalf_psum[:], lhsT=halfones[:], rhs=mlo[:],
                    start=True, stop=True,
                )
                # hi' = b_(j+1) if j < 64 else hi; lo' = b_j (bit-identical
                # fl ops to the bnd computation keep the invariant exact).
                nc.vector.tensor_scalar(
                    out=jp1[:], in0=jhalf_psum[:], scalar1=1.0, scalar2=None,
                    op0=Alu.add,
                )
                nc.vector.scalar_tensor_tensor(
                    out=hicand[:], in0=jp1[:], scalar=step[:], in1=lo[:],
                    op0=Alu.mult, op1=Alu.add,
                )
                nc.vector.tensor_scalar(
                    out=selhi[:], in0=jhalf_psum[:], scalar1=float(NB),
                    scalar2=None, op0=Alu.is_lt,
                )
                nc.vector.scalar_tensor_tensor(
                    out=lo[:], in0=jhalf_psum[:], scalar=step[:], in1=lo[:],
                    op0=Alu.mult, op1=Alu.add,
                )
                nc.vector.copy_predicated(
                    out=hi[:], mask=selhi[:], data=hicand[:]
                )

            # ---- 4b. binary tail (safety net; no-op once adjacent) ----
            for it in range(NTAIL):
                bs_junk = junkp.tile([P, NCHUNK], f32, tag="bsjunk")
                nc.vector.tensor_tensor(
                    out=mid[:], in0=lo[:], in1=hi[:], op=Alu.add
                )
                nc.vector.tensor_scalar_mul(mid[:], mid[:], 0.5)
                nc.vector.tensor_scalar(
                    out=bs_junk[:],
                    in0=scores_sb[:],
                    scalar1=mid[:],
                    scalar2=None,
                    op0=Alu.is_le,
                    op1=Alu.add,
                    accum_out=part[:],
                )
                cnt_psum = psum2.tile([P, 1], f32, space="PSUM", tag="cnt")
                nc.tensor.matmul(
                    out=cnt_psum[:], lhsT=ones128[:], rhs=part[:],
                    start=True, stop=True,
                )
                nc.vector.tensor_scalar(
                    out=pred[:], in0=cnt_psum[:], scalar1=float(K),
                    scalar2=None, op0=Alu.is_ge,
                )
                nc.vector.tensor_scalar(
                    out=npred[:], in0=cnt_psum[:], scalar1=float(K),
                    scalar2=None, op0=Alu.is_lt,
                )
                nc.vector.copy_predicated(out=hi[:], mask=pred[:], data=mid[:])
                nc.vector.copy_predicated(out=lo[:], mask=npred[:], data=mid[:])

            # ---- 5. exact selection mask with index tie-break ----
            thr16 = hi[:16, :1]
            F = N // 16  # 256
            mlt = smallp.tile([16, F], f32)
            red_lt = smallp.tile([16, 1], f32)
            nc.vector.tensor_scalar(
                out=mlt[:], in0=s1625[:], scalar1=thr16, scalar2=None,
                op0=Alu.is_lt, op1=Alu.add, accum_out=red_lt[:],
            )
            cnt16_psum = psum1.tile([16, 1], f32, space="PSUM")
            nc.tensor.matmul(
                out=cnt16_psum[:], lhsT=ones128[:16, :16],
                rhs=red_lt[:], start=True, stop=True,
            )
            # need = K - count_lt   (count of == t entries to keep)
            need16 = smallp.tile([16, 1], f32)
            nc.vector.tensor_scalar(
                out=need16[:], in0=cnt16_psum[:], scalar1=-1.0,
                scalar2=float(K), op0=Alu.mult, op1=Alu.add,
            )
            meq = smallp.tile([16, F], f32)
            nc.vector.tensor_scalar(
                out=meq[:], in0=s1625[:], scalar1=thr16, scalar2=None,
                op0=Alu.is_equal,
            )
            # global inclusive prefix of meq in f-major (logical n) order:
            #   colsum[f]  = sum_p meq[p, f]          (replicated, PE)
            #   colcum[f]  = inclusive scan_f colsum  (DVE scan)
            #   partial    = sum_{p'<=p} meq[p', f]   (PE, tri16 lhsT)
            #   prefix     = colcum - colsum + partial
            colsum_psum = psum1.tile([16, F], f32, space="PSUM")
            nc.tensor.matmul(
                out=colsum_psum[:], lhsT=ones128[:16, :16],
                rhs=meq[:], start=True, stop=True,
            )
            zeros16 = smallp.tile([16, F], f32)
            nc.vector.memset(zeros16[:], 0.0)
            colcum = smallp.tile([16, F], f32)
            nc.vector.tensor_tensor_scan(
                out=colcum[:], data0=zeros16[:], data1=colsum_psum[:],
                initial=0.0, op0=Alu.add, op1=Alu.add,
            )
            partial_psum = psum1.tile([16, F], f32, space="PSUM")
            nc.tensor.matmul(
                out=partial_psum[:], lhsT=tri16[:], rhs=meq[:],
                start=True, stop=True,
            )
            excl = smallp.tile([16, F], f32)
            nc.vector.tensor_tensor(
                out=excl[:], in0=colcum[:], in1=colsum_psum[:], op=Alu.subtract
            )
            prefix = smallp.tile([16, F], f32)
            nc.vector.tensor_tensor(
                out=prefix[:], in0=excl[:], in1=partial_psum[:], op=Alu.add
            )
            keepeq = smallp.tile([16, F], f32)
            nc.vector.tensor_scalar(
                out=keepeq[:], in0=prefix[:], scalar1=need16[:], scalar2=None,
                op0=Alu.is_le,
            )
            # select only where s == t:
            keepeq2 = smallp.tile([16, F], f32)
            nc.vector.tensor_tensor(
                out=keepeq2[:], in0=keepeq[:], in1=meq[:], op=Alu.mult
            )
            fmask = smallp.tile([16, F], f32)
            nc.vector.tensor_tensor(
                out=fmask[:], in0=mlt[:], in1=keepeq2[:], op=Alu.add
            )
            # masked_idx = fmask * (n+1) - 1   (selected -> n, else -1)
            masked = smallp.tile([16, F], f32)
            nc.vector.tensor_tensor(
                out=masked[:], in0=fmask[:], in1=iota_p1[:], op=Alu.mult
            )
            nc.vector.tensor_scalar(
                out=masked[:], in0=masked[:], scalar1=-1.0, scalar2=None,
                op0=Alu.add,
            )

            # ---- 6. compact to sorted index list ----
            compact = smallp.tile([16, K // 16], f32)
            nf = smallp.tile([1, 1], dt.uint32)
            nc.gpsimd.sparse_gather(
                out=compact[:], in_=masked[:], num_found=nf[:]
            )
            nc.sync.dma_start(out=nf_dbg[:, :], in_=nf[:])
            compact_i32 = smallp.tile([16, K // 16], dt.int32)
            nc.vector.tensor_copy(out=compact_i32[:], in_=compact[:])
            # Contiguous dump: d2[q*128 + f] = idx[f*16 + q]. Column m of
            # d2 (contiguous 512B) holds idx[j*16 + m] for j = 0..127, so
            # gather instruction m writes output rows j*16 + m (a stride-16
            # row view -- still whole 16KB/2KB rows per descriptor).
            idx_bounce = dramp.tile([K], dt.int32)
            nc.sync.dma_start(
                out=idx_bounce[:].rearrange("(q f) -> q f", q=16),
                in_=compact_i32[:],
            )
            nc.sync.dma_start(
                out=idx_dbg.rearrange("(q f) -> q f", q=16),
                in_=compact_i32[:],
            )
            adj_rows = new_adj.rearrange("(f q) w -> q f w", q=16)
            idxcols = []
            for m in range(16):
                idxcol = smallp.tile(
                    [P, 1], dt.int32, tag=f"idxcol{m}", name=f"idxcol{m}"
                )
                eng = nc.sync if m % 2 == 0 else nc.scalar
                eng.dma_start(
                    out=idxcol[:], in_=idx_bounce[m * P:(m + 1) * P, None]
                )
                idxcols.append(idxcol)
            h_rows = new_h.rearrange("(f q) w -> q f w", q=16)
            for m in range(16):
                adjrow = adjp.tile([P, N], f32)
                nc.gpsimd.indirect_dma_start(
                    out=adjrow[:],
                    out_offset=None,
                    in_=adj[:, :],
                    in_offset=bass.IndirectOffsetOnAxis(
                        ap=idxcols[m][:], axis=0
                    ),
                )
                eng = nc.sync if m % 2 == 0 else nc.scalar
                eng.dma_start(out=adj_rows[m], in_=adjrow[:])
                hrow = hrowp.tile([P, D], f32)
                nc.gpsimd.indirect_dma_start(
                    out=hrow[:],
                    out_offset=None,
                    in_=h[:, :],
                    in_offset=bass.IndirectOffsetOnAxis(
                        ap=idxcols[m][:], axis=0
                    ),
                )
                eng2 = nc.scalar if m % 2 == 0 else nc.sync
                eng2.dma_start(out=h_rows[m], in_=hrow[:])

    nc.compile()
    return nc


def _get_nc():
    if "nc" not in _cache:
        _cache["nc"] = _build_nc()
    return _cache["nc"]


def kernel(h, adj):
    from concourse.bass_utils import run_bass_kernel_spmd

    h = np.ascontiguousarray(np.asarray(h), dtype=np.float32)
    adj = np.ascontiguousarray(np.asarray(adj), dtype=np.float32)
    assert h.shape == (B, N, D) and adj.shape == (B, N, N)

    nc = _get_nc()
    in_maps = [{"h": h[b], "adj": adj[b]} for b in range(B)]
    res = run_bass_kernel_spmd(nc, in_maps, core_ids=list(range(B)))
    new_h = np.stack([res.results[b]["new_h"] for b in range(B)])
    new_adj = np.stack([res.results[b]["new_adj"] for b in range(B)])
    return new_h, new_adj
